# revision 1
# baseline (speedup 1.0000x reference)
"""Trainium2 Bass kernel for nn_BasicBlock (gnn_message_passing).

Sharding: 8 cores = (batch b in 0..4) x (half h in 0..2). Each core owns
N/2 = 16384 columns of one batch. Gathers run on-device via SWDGE
dma_gather in transpose mode against an SBUF-resident, token-wrapped
bf16 feature table (x for layer 1, out1 for layer 2). BatchNorm stats
are AllReduced across all 8 cores; out1 halves are exchanged between
the two cores of a batch with a pair AllGather.
"""
import sys
sys.path.insert(0, '/opt/trn_rl_repo')
import numpy as np
import ml_dtypes

B, C, N, K, KS = 4, 128, 32768, 9, 5
M = N // 2
ME = M + 4
CH = 512
NCHUNK = M // CH + 1          # 32 full + 1 overlap tail covering ME
NST = M // CH                 # owned super-tiles
NC_ = 8
EPS = 1e-5
BF16 = ml_dtypes.bfloat16
IDX_S = (K * CH) // 16        # idx cols per half-gather pair = full chunk wrap
HUGE = 1.0e4

_CACHE = {}
STAGE = 5
NCHUNK_RUN = 0
PARTS = 15
SP = False
GS = 1


def _build_program():
    import concourse.bacc as bacc
    import concourse.mybir as mybir
    import concourse.tile as tile
    from concourse.masks import make_identity

    f32 = mybir.dt.float32
    bf16 = mybir.dt.bfloat16
    i16 = mybir.dt.int16
    AF = mybir.ActivationFunctionType
    OP = mybir.AluOpType

    nc = bacc.Bacc("TRN2", target_bir_lowering=False, debug=False,
                   num_devices=NC_)

    # ---------------- external I/O ----------------
    xt_d = nc.dram_tensor("xt", [N, 128], bf16, kind="ExternalInput")
    idx_d = nc.dram_tensor("idx", [NCHUNK, 128, K * CH // 16], i16,
                           kind="ExternalInput")
    cs_d = nc.dram_tensor("cs", [15, M], f32, kind="ExternalInput")
    cc_d = nc.dram_tensor("cc", [15, M], f32, kind="ExternalInput")
    xres_d = nc.dram_tensor("xres", [128, M], f32, kind="ExternalInput")
    w1t_d = nc.dram_tensor("w1t", [128, K, 128], bf16, kind="ExternalInput")
    wc1t_d = nc.dram_tensor("wc1t", [128, KS, 128], bf16, kind="ExternalInput")
    w2t_d = nc.dram_tensor("w2t", [128, K, 128], bf16, kind="ExternalInput")
    wc2t_d = nc.dram_tensor("wc2t", [128, KS, 128], bf16, kind="ExternalInput")
    rep5_d = nc.dram_tensor("rep5", [5, KS, 128], bf16, kind="ExternalInput")
    s15_d = nc.dram_tensor("s15", [15, KS], bf16, kind="ExternalInput")
    gb_d = nc.dram_tensor("gb", [128, 8], f32, kind="ExternalInput")
    out_d = nc.dram_tensor("out", [128, M], f32, kind="ExternalOutput")

    with tile.TileContext(nc) as tc:
        with tc.tile_pool(name="persist", bufs=1) as pp, \
             tc.tile_pool(name="work", bufs=1) as wp, \
             tc.tile_pool(name="psum", bufs=1, space="PSUM") as ps, \
             tc.tile_pool(name="dram", bufs=1, space="DRAM") as dp:

            # ------------- persistent state -------------
            y1raw = pp.tile([128, ME], bf16)          # conv2d out (pre-BN)
            y2raw_t = pp.tile([128, M], bf16)         # wconv out scratch
            gw = pp.tile([5, M], bf16)                # gaussian weights
            w1t_t = pp.tile([128, K, 128], bf16)
            wc1t_t = pp.tile([128, KS, 128], bf16)
            w2t_t = pp.tile([128, K, 128], bf16)
            wc2t_t = pp.tile([128, KS, 128], bf16)
            rep5_t = pp.tile([5, KS, 128], bf16)
            s15_t = pp.tile([15, KS], bf16)
            gb_t = pp.tile([128, 8], f32)
            ident = pp.tile([128, 128], bf16)
            parts = [pp.tile([128, NCHUNK, 6], f32, name="parts0"),
                     pp.tile([128, NST, 6], f32, name="parts1"),
                     pp.tile([128, NCHUNK, 6], f32, name="parts2"),
                     pp.tile([128, NST, 6], f32, name="parts3")]
            stv = pp.tile([128, 8], f32)              # s1 t1 s2 t2 s3 t3 s4 t4

            nc.sync.dma_start(w1t_t[:], w1t_d[:])
            nc.sync.dma_start(wc1t_t[:], wc1t_d[:])
            nc.sync.dma_start(w2t_t[:], w2t_d[:])
            nc.sync.dma_start(wc2t_t[:], wc2t_d[:])
            nc.sync.dma_start(rep5_t[:], rep5_d[:])
            nc.sync.dma_start(s15_t[:], s15_d[:])
            nc.sync.dma_start(gb_t[:], gb_d[:])
            make_identity(nc, ident[:])

            yraw2 = y2raw_t[:]

            # DRAM bounce buffers
            d_my = dp.tile([M, 128], bf16)
            d_all = dp.tile([N, 128], bf16)
            ar_in = [dp.tile([128, 2], f32, name=f"ari{i}") for i in range(4)]
            ar_out = [dp.tile([128, 2], f32, name=f"aro{i}") for i in range(4)]

            def chunk_lo(st):
                return st * CH if st < NCHUNK - 1 else ME - CH

            def owned_slice(st):
                # owned ext-cols are [2, 2+M); chunk covers [lo, lo+CH)
                lo = chunk_lo(st)
                if st == 0:
                    return 2, CH
                if st < NCHUNK - 1:
                    return 0, CH
                return M - lo, M + 2 - lo              # tail: 2 cols

            # =====================================================
            def conv_gather_phase(src, wt_t, part, do_gw):
                for st in range(NCHUNK_RUN or NCHUNK):
                    idx_t = wp.tile([128, K * CH // 16], i16, tag="idx",
                                    bufs=3, name="idx_t")
                    nc.sync.dma_start(idx_t[:], idx_d[st])
                    py = ps.tile([128, CH], f32, tag="py", bufs=2, name="py")
                    if not (PARTS & 8):
                        nc.vector.memset(py[:], 0.0)
                    if GS == 1:
                        splits = [(0, 9)]
                    elif GS == 2:
                        splits = [(0, 5), (5, 4)]
                    else:
                        splits = [(k, 1) for k in range(K)]
                    for (base_k, nk) in (splits if PARTS & 1 else []):
                        g_t = wp.tile([128, 1, (9 if GS == 1 else 5) * CH],
                                      bf16, tag="g", bufs=4, name="g_t")
                        nidx = nk * CH
                        nc.gpsimd.dma_gather(
                            g_t[:, :, 0:nidx], src[:],
                            idx_t[:, base_k * CH // 16: (base_k + nk) * CH // 16],
                            nidx, nidx, 128, transpose=True,
                            single_packet=SP,
                        )
                        for k in range(nk if PARTS & 8 else 0):
                            kk = base_k + k
                            nc.tensor.matmul(
                                py[:], wt_t[:, kk, :],
                                g_t[:, 0, k * CH:(k + 1) * CH],
                                start=(kk == 0), stop=(kk == K - 1),
                            )
                    lo = chunk_lo(st)
                    nc.scalar.activation(y1raw[:, lo:lo + CH], py[:], AF.Copy)
                    if PARTS & 2:
                        a, b_ = owned_slice(st)
                        nc.vector.bn_stats(part[:, st, :], py[:, a:b_])
                    if (PARTS & 4) and do_gw and st < NST:
                        cs_t = wp.tile([15, CH], f32, tag="cs", bufs=4,
                                       name="cs_t")
                        cc_t = wp.tile([15, CH], f32, tag="cs", bufs=4,
                                       name="cc_t")
                        nc.sync.dma_start(cs_t[:], cs_d[:, st * CH:(st + 1) * CH])
                        nc.sync.dma_start(cc_t[:], cc_d[:, st * CH:(st + 1) * CH])
                        df_t = wp.tile([15, CH], bf16, tag="df", bufs=2,
                                       name="df_t")
                        nc.vector.tensor_tensor(out=df_t[:], in0=cs_t[:],
                                                in1=cc_t[:], op=OP.subtract)
                        sq_t = wp.tile([15, CH], bf16, tag="df", bufs=2,
                                       name="sq_t")
                        nc.vector.tensor_tensor(out=sq_t[:], in0=df_t[:],
                                                in1=df_t[:], op=OP.mult)
                        pd = ps.tile([5, CH], f32, tag="py", bufs=2, name="pd")
                        nc.tensor.matmul(pd[:], s15_t[:], sq_t[:],
                                         start=True, stop=True)
                        nc.scalar.activation(gw[:, st * CH:(st + 1) * CH],
                                             pd[:], AF.Exp, scale=-0.5)

            # =====================================================
            def stats_phase(part, gi, si, cid):
                """bn partials -> AllReduce -> scale/bias into stv[:, si:si+2]"""
                agg = wp.tile([128, 2], f32, tag="st2", bufs=8, name="agg")
                nc.vector.bn_aggr(agg[:], part[:])
                msq = wp.tile([128, 1], f32, tag="st1", bufs=16, name="msq")
                nc.vector.tensor_tensor(out=msq[:], in0=agg[:, 0:1],
                                        in1=agg[:, 0:1], op=OP.mult)
                ari = wp.tile([128, 2], f32, tag="st2", bufs=8, name="ari")
                nc.vector.tensor_copy(ari[:, 0:1], agg[:, 0:1])
                nc.vector.tensor_tensor(out=ari[:, 1:2], in0=agg[:, 1:2],
                                        in1=msq[:], op=OP.add)
                nc.sync.dma_start(ar_in[cid][:], ari[:])
                nc.gpsimd.collective_compute(
                    "AllReduce", OP.add,
                    replica_groups=[list(range(NC_))],
                    ins=[ar_in[cid].opt()], outs=[ar_out[cid].opt()],
                )
                ars = wp.tile([128, 2], f32, tag="st2", bufs=8, name="ars")
                nc.sync.dma_start(ars[:], ar_out[cid][:])
                pm = wp.tile([128, 2], f32, tag="st2", bufs=8, name="pm")
                nc.vector.tensor_scalar(out=pm[:], in0=ars[:],
                                        scalar1=1.0 / NC_, scalar2=None,
                                        op0=OP.mult)
                m2 = wp.tile([128, 1], f32, tag="st1", bufs=16, name="m2")
                nc.vector.tensor_tensor(out=m2[:], in0=pm[:, 0:1],
                                        in1=pm[:, 0:1], op=OP.mult)
                var = wp.tile([128, 1], f32, tag="st1", bufs=16, name="var")
                nc.vector.tensor_tensor(out=var[:], in0=pm[:, 1:2],
                                        in1=m2[:], op=OP.subtract)
                vpe = wp.tile([128, 1], f32, tag="st1", bufs=16, name="vpe")
                nc.vector.tensor_scalar(out=vpe[:], in0=var[:],
                                        scalar1=float(EPS), scalar2=None,
                                        op0=OP.add)
                sd = wp.tile([128, 1], f32, tag="st1", bufs=16, name="sd")
                nc.scalar.activation(sd[:], vpe[:], AF.Sqrt)
                rs = wp.tile([128, 1], f32, tag="st1", bufs=16, name="rs")
                nc.vector.reciprocal(rs[:], sd[:])
                nc.vector.tensor_tensor(out=stv[:, si:si + 1],
                                        in0=gb_t[:, gi:gi + 1], in1=rs[:],
                                        op=OP.mult)
                ms = wp.tile([128, 1], f32, tag="st1", bufs=16, name="ms")
                nc.vector.tensor_tensor(out=ms[:], in0=pm[:, 0:1],
                                        in1=stv[:, si:si + 1], op=OP.mult)
                nc.vector.tensor_tensor(out=stv[:, si + 1:si + 2],
                                        in0=gb_t[:, gi + 1:gi + 2], in1=ms[:],
                                        op=OP.subtract)

            # =====================================================
            def bn_relu_pass(si):
                for st in range(NST + 1):
                    lo = st * CH
                    w = CH if st < NST else ME - M
                    nc.scalar.activation(
                        y1raw[:, lo:lo + w], y1raw[:, lo:lo + w], AF.Relu,
                        bias=stv[:, si + 1:si + 2], scale=stv[:, si:si + 1])

            # =====================================================
            def wconv_phase(wct_t, part):
                for st in range(NST):
                    base = st * CH
                    py2 = ps.tile([128, CH], f32, tag="py", bufs=2, name="py2")
                    for ks in range(KS):
                        pgw = ps.tile([128, CH], f32, tag="pgw", bufs=5,
                                      name="pgw")
                        nc.tensor.matmul(pgw[:], rep5_t[:, ks, :],
                                         gw[:, base:base + CH],
                                         start=True, stop=True)
                        u_t = wp.tile([128, CH], bf16, tag="u", bufs=6,
                                      name="u_t")
                        nc.vector.tensor_tensor(
                            out=u_t[:], in0=pgw[:],
                            in1=y1raw[:, base + ks:base + ks + CH],
                            op=OP.mult)
                        nc.tensor.matmul(py2[:], wct_t[:, ks, :], u_t[:],
                                         start=(ks == 0), stop=(ks == KS - 1))
                    nc.vector.bn_stats(part[:, st, :], py2[:])
                    nc.scalar.activation(yraw2[:, base:base + CH], py2[:],
                                         AF.Copy)

            # =====================================================
            def dump(src_ap):
                # debug: cast-copy an AP [128, M] to the f32 output
                for st in range(NST):
                    sl = slice(st * CH, (st + 1) * CH)
                    nc.gpsimd.dma_start(out_d[:, sl], src_ap[:, sl])

            # layer 1
            if STAGE == 0:
                dump(yraw2)
            if STAGE >= 1:
                conv_gather_phase(xt_d, w1t_t, parts[0], do_gw=True)
            if STAGE == 1:
                dump(y1raw[:, 2:M + 2])
            if STAGE >= 2:
                stats_phase(parts[0], 0, 0, 0)
                bn_relu_pass(0)
            if STAGE == 2:
                dump(y1raw[:, 2:M + 2])
            if STAGE >= 3:
                wconv_phase(wc1t_t, parts[1])
            if STAGE == 3:
                dump(yraw2)
            if STAGE >= 4:
                stats_phase(parts[1], 2, 2, 1)

            # affine+relu, transpose, store halves, exchange
            for st in range(NST if STAGE >= 4 else 0):
                base = st * CH
                tmp = wp.tile([128, CH], bf16, tag="y2n", bufs=2, name="tmp")
                nc.scalar.activation(tmp[:], yraw2[:, base:base + CH], AF.Relu,
                                     bias=stv[:, 3:4], scale=stv[:, 2:3])
                stage = wp.tile([128, CH], bf16, tag="stage", bufs=2,
                                name="stage")
                for t4 in range(4):
                    ptr = ps.tile([128, 128], bf16, tag="py", bufs=2,
                                  name="ptr")
                    nc.tensor.transpose(ptr[:], tmp[:, t4 * 128:(t4 + 1) * 128],
                                        ident[:])
                    nc.vector.tensor_copy(stage[:, t4 * 128:(t4 + 1) * 128],
                                          ptr[:])
                nc.sync.dma_start(
                    d_my[base:base + CH, :].rearrange("(t p) o -> p t o", p=128),
                    stage[:].rearrange("p (t o) -> p t o", o=128))
            if STAGE >= 4:
                nc.gpsimd.collective_compute(
                    "AllGather", mybir.AluOpType.bypass,
                    replica_groups=[[2 * i, 2 * i + 1] for i in range(NC_ // 2)],
                    ins=[d_my.opt()], outs=[d_all.opt()],
                )
            if STAGE == 4:
                dump(yraw2)

            # layer 2
            if STAGE >= 5:
                conv_gather_phase(d_all, w2t_t, parts[2], do_gw=False)
                stats_phase(parts[2], 4, 4, 2)
                bn_relu_pass(4)
                wconv_phase(wc2t_t, parts[3])
                stats_phase(parts[3], 6, 6, 3)

            # final: out = relu(bn4(y4) + xres)
            for st in range(NST if STAGE >= 5 else 0):
                base = st * CH
                xr = wp.tile([128, CH], f32, tag="fw", bufs=4, name="xr")
                nc.sync.dma_start(xr[:], xres_d[:, base:base + CH])
                z = wp.tile([128, CH], f32, tag="fw", bufs=4, name="z")
                nc.vector.tensor_scalar(out=z[:], in0=yraw2[:, base:base + CH],
                                        scalar1=stv[:, 6:7],
                                        scalar2=stv[:, 7:8],
                                        op0=OP.mult, op1=OP.add)
                nc.vector.tensor_tensor(out=z[:], in0=z[:], in1=xr[:],
                                        op=OP.add)
                of = wp.tile([128, CH], f32, tag="fw", bufs=4, name="of")
                nc.scalar.activation(of[:], z[:], AF.Relu)
                nc.sync.dma_start(out_d[:, base:base + CH], of[:])

    nc.compile()
    return nc


def _wrap_idx(flat):
    """index i -> partition i%16, col i//16, replicated x8."""
    S = len(flat) // 16
    t16 = flat.astype(np.int16).reshape(S, 16).T
    return np.tile(t16, (8, 1))


def _prep_core_inputs(core, x, edge_index, coords, w1t, wc1t, w2t, wc2t,
                      rep5, s15, gbs):
    b, h = core // 2, core % 2
    xb = np.asarray(x[b], np.float32)                   # [C, N]
    xt = np.ascontiguousarray(xb.T).astype(BF16)        # [N, C]

    ei = np.asarray(edge_index[b])                      # [N, K]
    idx_chunks = np.zeros((NCHUNK, 128, K * CH // 16), np.int16)
    for st in range(NCHUNK):
        lo = st * CH if st < NCHUNK - 1 else ME - CH
        j = np.arange(lo, lo + CH)
        n = h * M - 2 + j
        valid = (n >= 0) & (n < N)
        nn = np.where(valid, n, 0)
        arr = ei[nn, :].T.astype(np.int16)              # [K, CH] k-major
        idx_chunks[st] = _wrap_idx(arr.reshape(-1))

    cb = np.asarray(coords[b], np.float32)              # [3, N]
    padded = np.full((3, N + 4), HUGE, np.float32)
    padded[:, 2:N + 2] = cb
    cs = np.empty((15, M), np.float32)
    for ks in range(KS):
        cs[ks * 3:(ks + 1) * 3] = padded[:, h * M + ks: h * M + ks + M]
    cc = np.tile(cb[:, h * M:(h + 1) * M], (KS, 1))

    return dict(
        xt=xt, idx=idx_chunks, cs=cs, cc=cc,
        xres=np.ascontiguousarray(xb[:, h * M:(h + 1) * M]),
        w1t=w1t, wc1t=wc1t, w2t=w2t, wc2t=wc2t,
        rep5=rep5, s15=s15, gb=gbs,
    )


def kernel(**inputs):
    from concourse import bass_utils

    if "nc" not in _CACHE:
        _CACHE["nc"] = _build_program()
    nc = _CACHE["nc"]

    x = np.asarray(inputs["x"], np.float32)
    edge_index = np.asarray(inputs["edge_index"])
    coords = np.asarray(inputs["coords"], np.float32)

    w1t = np.ascontiguousarray(
        np.transpose(np.asarray(inputs["w2d_1"], np.float32), (1, 2, 0))
    ).astype(BF16)
    wc1t = np.ascontiguousarray(
        np.transpose(np.asarray(inputs["wc_1"], np.float32), (1, 2, 0))
    ).astype(BF16)
    w2t = np.ascontiguousarray(
        np.transpose(np.asarray(inputs["w2d_2"], np.float32), (1, 2, 0))
    ).astype(BF16)
    wc2t = np.ascontiguousarray(
        np.transpose(np.asarray(inputs["wc_2"], np.float32), (1, 2, 0))
    ).astype(BF16)
    rep5 = np.zeros((5, KS, 128), np.float32)
    for ks in range(KS):
        rep5[ks, ks, :] = 1.0
    rep5 = rep5.astype(BF16)
    s15 = np.zeros((15, KS), np.float32)
    for r in range(15):
        s15[r, r // 3] = 1.0
    s15 = s15.astype(BF16)
    gbs = np.stack([
        np.asarray(inputs["g2d_1"], np.float32),
        np.asarray(inputs["b2d_1"], np.float32),
        np.asarray(inputs["g1d_1"], np.float32),
        np.asarray(inputs["b1d_1"], np.float32),
        np.asarray(inputs["g2d_2"], np.float32),
        np.asarray(inputs["b2d_2"], np.float32),
        np.asarray(inputs["g1d_2"], np.float32),
        np.asarray(inputs["b1d_2"], np.float32),
    ], axis=1)

    in_maps = [
        _prep_core_inputs(c, x, edge_index, coords, w1t, wc1t, w2t, wc2t,
                          rep5, s15, gbs)
        for c in range(NC_)
    ]
    res = bass_utils.run_bass_kernel_spmd(
        nc, in_maps, core_ids=list(range(NC_)),
        trace=_CACHE.get("trace", False),
    )
    _CACHE["last_results"] = res

    out = np.empty((B, C, N), np.float32)
    for c in range(NC_):
        b, h = c // 2, c % 2
        out[b, :, h * M:(h + 1) * M] = res.results[c]["out"]
    return out



# revision 8
# speedup vs baseline: 1.5440x; 1.5440x over previous
"""Trainium2 Bass kernel for nn_BasicBlock (gnn_message_passing).

Sharding: 8 cores = (batch b in 0..4) x (half h in 0..2). Each core owns
N/2 = 16384 columns of one batch. Gathers run on-device via SWDGE
dma_gather in transpose mode against an SBUF-resident, token-wrapped
bf16 feature table (x for layer 1, out1 for layer 2). BatchNorm stats
are AllReduced across all 8 cores; out1 halves are exchanged between
the two cores of a batch with a pair AllGather.
"""
import sys
sys.path.insert(0, '/opt/trn_rl_repo')
import numpy as np
import ml_dtypes

B, C, N, K, KS = 4, 128, 32768, 9, 5
M = N // 2
ME = M + 4
CH = 512
NCHUNK = M // CH + 1          # 32 full + 1 overlap tail covering ME
NST = M // CH                 # owned super-tiles
NC_ = 8
EPS = 1e-5
BF16 = ml_dtypes.bfloat16
IDX_S = (K * CH) // 16        # idx cols per half-gather pair = full chunk wrap
HUGE = 1.0e4

_CACHE = {}
STAGE = 5
NCHUNK_RUN = 0
PARTS = 15
SP = False
GS = 1


def _build_program():
    import concourse.bacc as bacc
    import concourse.mybir as mybir
    import concourse.tile as tile
    from concourse.masks import make_identity

    f32 = mybir.dt.float32
    bf16 = mybir.dt.bfloat16
    i16 = mybir.dt.int16
    AF = mybir.ActivationFunctionType
    OP = mybir.AluOpType

    nc = bacc.Bacc("TRN2", target_bir_lowering=False, debug=False,
                   num_devices=NC_, num_swdge_queues=4)

    # ---------------- external I/O ----------------
    xt_d = nc.dram_tensor("xt", [N, 128], bf16, kind="ExternalInput")
    idx_d = nc.dram_tensor("idx", [NCHUNK, 128, K * CH // 16], i16,
                           kind="ExternalInput")
    cs_d = nc.dram_tensor("cs", [15, M], f32, kind="ExternalInput")
    cc_d = nc.dram_tensor("cc", [15, M], f32, kind="ExternalInput")
    xres_d = nc.dram_tensor("xres", [128, M], f32, kind="ExternalInput")
    w1t_d = nc.dram_tensor("w1t", [128, K, 128], bf16, kind="ExternalInput")
    wc1t_d = nc.dram_tensor("wc1t", [128, KS, 128], bf16, kind="ExternalInput")
    w2t_d = nc.dram_tensor("w2t", [128, K, 128], bf16, kind="ExternalInput")
    wc2t_d = nc.dram_tensor("wc2t", [128, KS, 128], bf16, kind="ExternalInput")
    rep5_d = nc.dram_tensor("rep5", [5, KS, 128], bf16, kind="ExternalInput")
    s15_d = nc.dram_tensor("s15", [15, KS], bf16, kind="ExternalInput")
    gb_d = nc.dram_tensor("gb", [128, 8], f32, kind="ExternalInput")
    out_d = nc.dram_tensor("out", [128, M], f32, kind="ExternalOutput")

    with tile.TileContext(nc) as tc:
        with tc.tile_pool(name="persist", bufs=1) as pp, \
             tc.tile_pool(name="work", bufs=1) as wp, \
             tc.tile_pool(name="psum", bufs=1, space="PSUM") as ps, \
             tc.tile_pool(name="dram", bufs=1, space="DRAM") as dp:

            # ------------- persistent state -------------
            y1raw = pp.tile([128, ME], bf16)          # conv2d out (pre-BN)
            y2raw_t = pp.tile([128, M], bf16)         # wconv out scratch
            gw = pp.tile([5, M], bf16)                # gaussian weights
            w1t_t = pp.tile([128, K, 128], bf16)
            wc1t_t = pp.tile([128, KS, 128], bf16)
            w2t_t = pp.tile([128, K, 128], bf16)
            wc2t_t = pp.tile([128, KS, 128], bf16)
            rep5_t = pp.tile([5, KS, 128], bf16)
            s15_t = pp.tile([15, KS], bf16)
            gb_t = pp.tile([128, 8], f32)
            ident = pp.tile([128, 128], bf16)
            parts = [pp.tile([128, NCHUNK, 6], f32, name="parts0"),
                     pp.tile([128, NST, 6], f32, name="parts1"),
                     pp.tile([128, NCHUNK, 6], f32, name="parts2"),
                     pp.tile([128, NST, 6], f32, name="parts3")]
            stv = pp.tile([128, 8], f32)              # s1 t1 s2 t2 s3 t3 s4 t4

            nc.sync.dma_start(w1t_t[:], w1t_d[:])
            nc.sync.dma_start(wc1t_t[:], wc1t_d[:])
            nc.sync.dma_start(w2t_t[:], w2t_d[:])
            nc.sync.dma_start(wc2t_t[:], wc2t_d[:])
            nc.sync.dma_start(rep5_t[:], rep5_d[:])
            nc.sync.dma_start(s15_t[:], s15_d[:])
            nc.sync.dma_start(gb_t[:], gb_d[:])
            make_identity(nc, ident[:])

            yraw2 = y2raw_t[:]

            # DRAM bounce buffers
            d_my = dp.tile([M, 128], bf16)
            d_all = dp.tile([N, 128], bf16)
            ar_in = [dp.tile([128, 2], f32, name=f"ari{i}") for i in range(4)]
            ar_out = [dp.tile([128, 2], f32, name=f"aro{i}") for i in range(4)]

            gather_ctr = [0]

            def chunk_lo(st):
                return st * CH if st < NCHUNK - 1 else ME - CH

            def owned_slice(st):
                # owned ext-cols are [2, 2+M); chunk covers [lo, lo+CH)
                lo = chunk_lo(st)
                if st == 0:
                    return 2, CH
                if st < NCHUNK - 1:
                    return 0, CH
                return M - lo, M + 2 - lo              # tail: 2 cols

            # =====================================================
            # chunk plan: which chunks use transpose-mode gather (queue 0)
            # vs row-mode gather (queues 1-3) + on-chip PE transpose.
            T_COUNT = 11
            t_pos = {round(i * NCHUNK / T_COUNT) for i in range(T_COUNT)}
            CHUNK_PLAN = []
            _rq = 0
            for _st in range(NCHUNK):
                if _st in t_pos:
                    CHUNK_PLAN.append((True, 0))
                else:
                    _rq = _rq % 3 + 1
                    CHUNK_PLAN.append((False, _rq))

            def conv_gather_phase(src, wt_t, part, do_gw):
                NI = K * CH
                for st in range(NCHUNK_RUN or NCHUNK):
                    is_t, q = CHUNK_PLAN[st]
                    idx_t = wp.tile([128, NI // 16], i16, tag="idx",
                                    bufs=6, name="idx_t")
                    nc.sync.dma_start(idx_t[:], idx_d[st])
                    py = ps.tile([128, CH], f32, tag="py", bufs=2, name="py")
                    g_t = wp.tile([128, 1, K * CH],
                                  bf16, tag="g", bufs=4, name="g_t")
                    if is_t:
                        nc.gpsimd.dma_gather(
                            g_t[:, :, 0:NI], src[:], idx_t[:],
                            NI, NI, 128, transpose=True,
                            single_packet=SP, queue_num=0,
                        )
                    else:
                        r3 = wp.tile([128, NI // 128, 128], bf16, tag="r3",
                                     bufs=3, name="r3")
                        nc.gpsimd.dma_gather(
                            r3[:], src[:], idx_t[:],
                            NI, NI, 128, transpose=False,
                            single_packet=SP, queue_num=q,
                        )
                        for m in range(K):
                            pt = ps.tile([128, CH], bf16, tag="pt", bufs=3,
                                         name="pt")
                            for t4 in range(4):
                                nc.tensor.transpose(
                                    pt[:, t4 * 128:(t4 + 1) * 128],
                                    r3[:, m * 4 + t4, :], ident[:])
                            nc.scalar.activation(
                                g_t[:, 0, m * CH:(m + 1) * CH], pt[:], AF.Copy)
                    for k in range(K):
                        nc.tensor.matmul(
                            py[:], wt_t[:, k, :],
                            g_t[:, 0, k * CH:(k + 1) * CH],
                            start=(k == 0), stop=(k == K - 1),
                        )
                    lo = chunk_lo(st)
                    nc.scalar.activation(y1raw[:, lo:lo + CH], py[:], AF.Copy)
                    if PARTS & 2:
                        a, b_ = owned_slice(st)
                        nc.vector.bn_stats(part[:, st, :], py[:, a:b_])
                    if (PARTS & 4) and do_gw and st < NST:
                        cs_t = wp.tile([15, CH], f32, tag="cs", bufs=4,
                                       name="cs_t")
                        cc_t = wp.tile([15, CH], f32, tag="cs", bufs=4,
                                       name="cc_t")
                        nc.sync.dma_start(cs_t[:], cs_d[:, st * CH:(st + 1) * CH])
                        nc.sync.dma_start(cc_t[:], cc_d[:, st * CH:(st + 1) * CH])
                        df_t = wp.tile([15, CH], bf16, tag="df", bufs=2,
                                       name="df_t")
                        nc.vector.tensor_tensor(out=df_t[:], in0=cs_t[:],
                                                in1=cc_t[:], op=OP.subtract)
                        sq_t = wp.tile([15, CH], bf16, tag="df", bufs=2,
                                       name="sq_t")
                        nc.vector.tensor_tensor(out=sq_t[:], in0=df_t[:],
                                                in1=df_t[:], op=OP.mult)
                        pd = ps.tile([5, CH], f32, tag="py", bufs=2, name="pd")
                        nc.tensor.matmul(pd[:], s15_t[:], sq_t[:],
                                         start=True, stop=True)
                        nc.scalar.activation(gw[:, st * CH:(st + 1) * CH],
                                             pd[:], AF.Exp, scale=-0.5)

            # =====================================================
            def stats_phase(part, gi, si, cid):
                """bn partials -> AllReduce -> scale/bias into stv[:, si:si+2]"""
                agg = wp.tile([128, 2], f32, tag="st2", bufs=8, name="agg")
                nc.vector.bn_aggr(agg[:], part[:])
                msq = wp.tile([128, 1], f32, tag="st1", bufs=16, name="msq")
                nc.vector.tensor_tensor(out=msq[:], in0=agg[:, 0:1],
                                        in1=agg[:, 0:1], op=OP.mult)
                ari = wp.tile([128, 2], f32, tag="st2", bufs=8, name="ari")
                nc.vector.tensor_copy(ari[:, 0:1], agg[:, 0:1])
                nc.vector.tensor_tensor(out=ari[:, 1:2], in0=agg[:, 1:2],
                                        in1=msq[:], op=OP.add)
                nc.sync.dma_start(ar_in[cid][:], ari[:])
                nc.gpsimd.collective_compute(
                    "AllReduce", OP.add,
                    replica_groups=[list(range(NC_))],
                    ins=[ar_in[cid].opt()], outs=[ar_out[cid].opt()],
                )
                ars = wp.tile([128, 2], f32, tag="st2", bufs=8, name="ars")
                nc.sync.dma_start(ars[:], ar_out[cid][:])
                pm = wp.tile([128, 2], f32, tag="st2", bufs=8, name="pm")
                nc.vector.tensor_scalar(out=pm[:], in0=ars[:],
                                        scalar1=1.0 / NC_, scalar2=None,
                                        op0=OP.mult)
                m2 = wp.tile([128, 1], f32, tag="st1", bufs=16, name="m2")
                nc.vector.tensor_tensor(out=m2[:], in0=pm[:, 0:1],
                                        in1=pm[:, 0:1], op=OP.mult)
                var = wp.tile([128, 1], f32, tag="st1", bufs=16, name="var")
                nc.vector.tensor_tensor(out=var[:], in0=pm[:, 1:2],
                                        in1=m2[:], op=OP.subtract)
                vpe = wp.tile([128, 1], f32, tag="st1", bufs=16, name="vpe")
                nc.vector.tensor_scalar(out=vpe[:], in0=var[:],
                                        scalar1=float(EPS), scalar2=None,
                                        op0=OP.add)
                sd = wp.tile([128, 1], f32, tag="st1", bufs=16, name="sd")
                nc.scalar.activation(sd[:], vpe[:], AF.Sqrt)
                rs = wp.tile([128, 1], f32, tag="st1", bufs=16, name="rs")
                nc.vector.reciprocal(rs[:], sd[:])
                nc.vector.tensor_tensor(out=stv[:, si:si + 1],
                                        in0=gb_t[:, gi:gi + 1], in1=rs[:],
                                        op=OP.mult)
                ms = wp.tile([128, 1], f32, tag="st1", bufs=16, name="ms")
                nc.vector.tensor_tensor(out=ms[:], in0=pm[:, 0:1],
                                        in1=stv[:, si:si + 1], op=OP.mult)
                nc.vector.tensor_tensor(out=stv[:, si + 1:si + 2],
                                        in0=gb_t[:, gi + 1:gi + 2], in1=ms[:],
                                        op=OP.subtract)

            # =====================================================
            def bn_relu_pass(si):
                for st in range(NST + 1):
                    lo = st * CH
                    w = CH if st < NST else ME - M
                    nc.scalar.activation(
                        y1raw[:, lo:lo + w], y1raw[:, lo:lo + w], AF.Relu,
                        bias=stv[:, si + 1:si + 2], scale=stv[:, si:si + 1])

            # =====================================================
            def wconv_phase(wct_t, part):
                for st in range(NST):
                    base = st * CH
                    py2 = ps.tile([128, CH], f32, tag="py", bufs=2, name="py2")
                    for ks in range(KS):
                        pgw = ps.tile([128, CH], f32, tag="pgw", bufs=2,
                                      name="pgw")
                        nc.tensor.matmul(pgw[:], rep5_t[:, ks, :],
                                         gw[:, base:base + CH],
                                         start=True, stop=True)
                        u_t = wp.tile([128, CH], bf16, tag="u", bufs=6,
                                      name="u_t")
                        nc.vector.tensor_tensor(
                            out=u_t[:], in0=pgw[:],
                            in1=y1raw[:, base + ks:base + ks + CH],
                            op=OP.mult)
                        nc.tensor.matmul(py2[:], wct_t[:, ks, :], u_t[:],
                                         start=(ks == 0), stop=(ks == KS - 1))
                    nc.vector.bn_stats(part[:, st, :], py2[:])
                    nc.scalar.activation(yraw2[:, base:base + CH], py2[:],
                                         AF.Copy)

            # =====================================================
            def dump(src_ap):
                # debug: cast-copy an AP [128, M] to the f32 output
                for st in range(NST):
                    sl = slice(st * CH, (st + 1) * CH)
                    nc.gpsimd.dma_start(out_d[:, sl], src_ap[:, sl])

            # layer 1
            if STAGE == 0:
                dump(yraw2)
            if STAGE >= 1:
                conv_gather_phase(xt_d, w1t_t, parts[0], do_gw=True)
            if STAGE == 1:
                dump(y1raw[:, 2:M + 2])
            if STAGE >= 2:
                stats_phase(parts[0], 0, 0, 0)
                bn_relu_pass(0)
            if STAGE == 2:
                dump(y1raw[:, 2:M + 2])
            if STAGE >= 3:
                wconv_phase(wc1t_t, parts[1])
            if STAGE == 3:
                dump(yraw2)
            if STAGE >= 4:
                stats_phase(parts[1], 2, 2, 1)

            # affine+relu, transpose, store halves, exchange
            for st in range(NST if STAGE >= 4 else 0):
                base = st * CH
                tmp = wp.tile([128, CH], bf16, tag="y2n", bufs=2, name="tmp")
                nc.scalar.activation(tmp[:], yraw2[:, base:base + CH], AF.Relu,
                                     bias=stv[:, 3:4], scale=stv[:, 2:3])
                stage = wp.tile([128, CH], bf16, tag="stage", bufs=2,
                                name="stage")
                for t4 in range(4):
                    ptr = ps.tile([128, 128], bf16, tag="py", bufs=2,
                                  name="ptr")
                    nc.tensor.transpose(ptr[:], tmp[:, t4 * 128:(t4 + 1) * 128],
                                        ident[:])
                    nc.vector.tensor_copy(stage[:, t4 * 128:(t4 + 1) * 128],
                                          ptr[:])
                nc.sync.dma_start(
                    d_my[base:base + CH, :].rearrange("(t p) o -> p t o", p=128),
                    stage[:].rearrange("p (t o) -> p t o", o=128))
            if STAGE >= 4:
                nc.gpsimd.collective_compute(
                    "AllGather", mybir.AluOpType.bypass,
                    replica_groups=[[2 * i, 2 * i + 1] for i in range(NC_ // 2)],
                    ins=[d_my.opt()], outs=[d_all.opt()],
                )
            if STAGE == 4:
                dump(yraw2)

            # layer 2
            if STAGE >= 5:
                conv_gather_phase(d_all, w2t_t, parts[2], do_gw=False)
                stats_phase(parts[2], 4, 4, 2)
                bn_relu_pass(4)
                wconv_phase(wc2t_t, parts[3])
                stats_phase(parts[3], 6, 6, 3)

            # final: out = relu(bn4(y4) + xres)
            for st in range(NST if STAGE >= 5 else 0):
                base = st * CH
                xr = wp.tile([128, CH], f32, tag="fw", bufs=4, name="xr")
                nc.sync.dma_start(xr[:], xres_d[:, base:base + CH])
                z = wp.tile([128, CH], f32, tag="fw", bufs=4, name="z")
                nc.vector.tensor_scalar(out=z[:], in0=yraw2[:, base:base + CH],
                                        scalar1=stv[:, 6:7],
                                        scalar2=stv[:, 7:8],
                                        op0=OP.mult, op1=OP.add)
                nc.vector.tensor_tensor(out=z[:], in0=z[:], in1=xr[:],
                                        op=OP.add)
                of = wp.tile([128, CH], f32, tag="fw", bufs=4, name="of")
                nc.scalar.activation(of[:], z[:], AF.Relu)
                nc.sync.dma_start(out_d[:, base:base + CH], of[:])

    nc.compile()
    return nc


def _wrap_idx(flat):
    """index i -> partition i%16, col i//16, replicated x8."""
    S = len(flat) // 16
    t16 = flat.astype(np.int16).reshape(S, 16).T
    return np.tile(t16, (8, 1))


def _prep_core_inputs(core, x, edge_index, coords, w1t, wc1t, w2t, wc2t,
                      rep5, s15, gbs):
    b, h = core // 2, core % 2
    xb = np.asarray(x[b], np.float32)                   # [C, N]
    xt = np.ascontiguousarray(xb.T).astype(BF16)        # [N, C]

    ei = np.asarray(edge_index[b])                      # [N, K]
    idx_chunks = np.zeros((NCHUNK, 128, K * CH // 16), np.int16)
    for st in range(NCHUNK):
        lo = st * CH if st < NCHUNK - 1 else ME - CH
        j = np.arange(lo, lo + CH)
        n = h * M - 2 + j
        valid = (n >= 0) & (n < N)
        nn = np.where(valid, n, 0)
        arr = ei[nn, :].T.astype(np.int16)              # [K, CH] k-major
        idx_chunks[st] = _wrap_idx(arr.reshape(-1))

    cb = np.asarray(coords[b], np.float32)              # [3, N]
    padded = np.full((3, N + 4), HUGE, np.float32)
    padded[:, 2:N + 2] = cb
    cs = np.empty((15, M), np.float32)
    for ks in range(KS):
        cs[ks * 3:(ks + 1) * 3] = padded[:, h * M + ks: h * M + ks + M]
    cc = np.tile(cb[:, h * M:(h + 1) * M], (KS, 1))

    return dict(
        xt=xt, idx=idx_chunks, cs=cs, cc=cc,
        xres=np.ascontiguousarray(xb[:, h * M:(h + 1) * M]),
        w1t=w1t, wc1t=wc1t, w2t=w2t, wc2t=wc2t,
        rep5=rep5, s15=s15, gb=gbs,
    )


def kernel(**inputs):
    from concourse import bass_utils

    if "nc" not in _CACHE:
        _CACHE["nc"] = _build_program()
    nc = _CACHE["nc"]

    x = np.asarray(inputs["x"], np.float32)
    edge_index = np.asarray(inputs["edge_index"])
    coords = np.asarray(inputs["coords"], np.float32)

    w1t = np.ascontiguousarray(
        np.transpose(np.asarray(inputs["w2d_1"], np.float32), (1, 2, 0))
    ).astype(BF16)
    wc1t = np.ascontiguousarray(
        np.transpose(np.asarray(inputs["wc_1"], np.float32), (1, 2, 0))
    ).astype(BF16)
    w2t = np.ascontiguousarray(
        np.transpose(np.asarray(inputs["w2d_2"], np.float32), (1, 2, 0))
    ).astype(BF16)
    wc2t = np.ascontiguousarray(
        np.transpose(np.asarray(inputs["wc_2"], np.float32), (1, 2, 0))
    ).astype(BF16)
    rep5 = np.zeros((5, KS, 128), np.float32)
    for ks in range(KS):
        rep5[ks, ks, :] = 1.0
    rep5 = rep5.astype(BF16)
    s15 = np.zeros((15, KS), np.float32)
    for r in range(15):
        s15[r, r // 3] = 1.0
    s15 = s15.astype(BF16)
    gbs = np.stack([
        np.asarray(inputs["g2d_1"], np.float32),
        np.asarray(inputs["b2d_1"], np.float32),
        np.asarray(inputs["g1d_1"], np.float32),
        np.asarray(inputs["b1d_1"], np.float32),
        np.asarray(inputs["g2d_2"], np.float32),
        np.asarray(inputs["b2d_2"], np.float32),
        np.asarray(inputs["g1d_2"], np.float32),
        np.asarray(inputs["b1d_2"], np.float32),
    ], axis=1)

    in_maps = [
        _prep_core_inputs(c, x, edge_index, coords, w1t, wc1t, w2t, wc2t,
                          rep5, s15, gbs)
        for c in range(NC_)
    ]
    res = bass_utils.run_bass_kernel_spmd(
        nc, in_maps, core_ids=list(range(NC_)),
        trace=_CACHE.get("trace", False),
    )
    _CACHE["last_results"] = res

    out = np.empty((B, C, N), np.float32)
    for c in range(NC_):
        b, h = c // 2, c % 2
        out[b, :, h * M:(h + 1) * M] = res.results[c]["out"]
    return out



# revision 13
# speedup vs baseline: 1.5953x; 1.0332x over previous
"""Trainium2 Bass kernel for nn_BasicBlock (gnn_message_passing).

Sharding: 8 cores = (batch b in 0..4) x (half h in 0..2). Each core owns
N/2 = 16384 columns of one batch. Gathers run on-device via SWDGE
dma_gather in transpose mode against an SBUF-resident, token-wrapped
bf16 feature table (x for layer 1, out1 for layer 2). BatchNorm stats
are AllReduced across all 8 cores; out1 halves are exchanged between
the two cores of a batch with a pair AllGather.
"""
import sys
sys.path.insert(0, '/opt/trn_rl_repo')
import numpy as np
import ml_dtypes

B, C, N, K, KS = 4, 128, 32768, 9, 5
M = N // 2
ME = M + 4
CH = 512
NCHUNK = M // CH + 1          # 32 full + 1 overlap tail covering ME
NST = M // CH                 # owned super-tiles
NC_ = 8
EPS = 1e-5
BF16 = ml_dtypes.bfloat16
IDX_S = (K * CH) // 16        # idx cols per half-gather pair = full chunk wrap
HUGE = 1.0e4

_CACHE = {}
STAGE = 5
NCHUNK_RUN = 0
PARTS = 15
SP = False
GS = 1


def _build_program():
    import concourse.bacc as bacc
    import concourse.mybir as mybir
    import concourse.tile as tile
    from concourse.masks import make_identity

    f32 = mybir.dt.float32
    bf16 = mybir.dt.bfloat16
    i16 = mybir.dt.int16
    AF = mybir.ActivationFunctionType
    OP = mybir.AluOpType

    nc = bacc.Bacc("TRN2", target_bir_lowering=False, debug=False,
                   num_devices=NC_, num_swdge_queues=4)

    # ---------------- external I/O ----------------
    xt_d = nc.dram_tensor("xt", [N, 128], bf16, kind="ExternalInput")
    idx_d = nc.dram_tensor("idx", [NCHUNK, 128, K * CH // 16], i16,
                           kind="ExternalInput")
    cs_d = nc.dram_tensor("cs", [15, M], f32, kind="ExternalInput")
    cc_d = nc.dram_tensor("cc", [15, M], f32, kind="ExternalInput")
    xres_d = nc.dram_tensor("xres", [128, M], f32, kind="ExternalInput")
    w1t_d = nc.dram_tensor("w1t", [128, K, 128], bf16, kind="ExternalInput")
    wc1t_d = nc.dram_tensor("wc1t", [128, KS, 128], bf16, kind="ExternalInput")
    w2t_d = nc.dram_tensor("w2t", [128, K, 128], bf16, kind="ExternalInput")
    wc2t_d = nc.dram_tensor("wc2t", [128, KS, 128], bf16, kind="ExternalInput")
    rep5_d = nc.dram_tensor("rep5", [5, KS, 128], bf16, kind="ExternalInput")
    s15_d = nc.dram_tensor("s15", [15, KS], bf16, kind="ExternalInput")
    gb_d = nc.dram_tensor("gb", [128, 8], f32, kind="ExternalInput")
    out_d = nc.dram_tensor("out", [128, M], f32, kind="ExternalOutput")

    with tile.TileContext(nc) as tc:
        with tc.tile_pool(name="persist", bufs=1) as pp, \
             tc.tile_pool(name="work", bufs=1) as wp, \
             tc.tile_pool(name="psum", bufs=1, space="PSUM") as ps, \
             tc.tile_pool(name="dram", bufs=1, space="DRAM") as dp:

            # ------------- persistent state -------------
            y1raw = pp.tile([128, ME], bf16)          # conv2d out (pre-BN)
            y2raw_t = pp.tile([128, M], bf16)         # wconv out scratch
            w1t_t = pp.tile([128, K, 128], bf16)
            wc1t_t = pp.tile([128, KS, 128], bf16)
            w2t_t = pp.tile([128, K, 128], bf16)
            wc2t_t = pp.tile([128, KS, 128], bf16)
            rep5_t = pp.tile([5, KS, 128], bf16)
            s15_t = pp.tile([15, KS], bf16)
            gb_t = pp.tile([128, 8], f32)
            ident = pp.tile([128, 128], bf16)
            parts = [pp.tile([128, NCHUNK, 6], f32, name="parts0"),
                     pp.tile([128, NST, 6], f32, name="parts1"),
                     pp.tile([128, NCHUNK, 6], f32, name="parts2"),
                     pp.tile([128, NST, 6], f32, name="parts3")]
            stv = pp.tile([128, 8], f32)              # s1 t1 s2 t2 s3 t3 s4 t4

            nc.sync.dma_start(w1t_t[:], w1t_d[:])
            nc.sync.dma_start(wc1t_t[:], wc1t_d[:])
            nc.sync.dma_start(w2t_t[:], w2t_d[:])
            nc.sync.dma_start(wc2t_t[:], wc2t_d[:])
            nc.sync.dma_start(rep5_t[:], rep5_d[:])
            nc.sync.dma_start(s15_t[:], s15_d[:])
            nc.sync.dma_start(gb_t[:], gb_d[:])
            make_identity(nc, ident[:])

            yraw2 = y2raw_t[:]

            # DRAM bounce buffers
            d_my = dp.tile([M, 128], bf16)
            d_all = dp.tile([N, 128], bf16)
            gw_d = dp.tile([5, M], bf16)              # gaussian weights (spilled)
            ar_in = [dp.tile([128, 2], f32, name=f"ari{i}") for i in range(4)]
            ar_out = [dp.tile([128, 2], f32, name=f"aro{i}") for i in range(4)]

            gather_ctr = [0]

            def chunk_lo(st):
                return st * CH if st < NCHUNK - 1 else ME - CH

            def owned_slice(st):
                # owned ext-cols are [2, 2+M); chunk covers [lo, lo+CH)
                lo = chunk_lo(st)
                if st == 0:
                    return 2, CH
                if st < NCHUNK - 1:
                    return 0, CH
                return M - lo, M + 2 - lo              # tail: 2 cols

            # =====================================================
            # chunk plan: which chunks use transpose-mode gather (queue 0)
            # vs row-mode gather (queues 1-3) + on-chip PE transpose.
            T_COUNT = 10
            t_pos = {round(i * NCHUNK / T_COUNT) for i in range(T_COUNT)}
            CHUNK_PLAN = []
            _rq = 0
            for _st in range(NCHUNK):
                if _st in t_pos:
                    CHUNK_PLAN.append((True, 0))
                else:
                    _rq = _rq % 3 + 1
                    CHUNK_PLAN.append((False, _rq))

            def conv_gather_phase(src, wt_t, part, do_gw):
                NI = K * CH
                for st in range(NCHUNK_RUN or NCHUNK):
                    is_t, q = CHUNK_PLAN[st]
                    idx_t = wp.tile([128, NI // 16], i16, tag="idx",
                                    bufs=6, name="idx_t")
                    nc.sync.dma_start(idx_t[:], idx_d[st])
                    py = ps.tile([128, CH], f32, tag="py", bufs=2, name="py")
                    g_t = wp.tile([128, 1, K * CH],
                                  bf16, tag="g", bufs=6, name="g_t")
                    if is_t:
                        nc.gpsimd.dma_gather(
                            g_t[:, :, 0:NI], src[:], idx_t[:],
                            NI, NI, 128, transpose=True,
                            single_packet=SP, queue_num=0,
                        )
                    else:
                        r3 = wp.tile([128, NI // 128, 128], bf16, tag="r3",
                                     bufs=3, name="r3")
                        nc.gpsimd.dma_gather(
                            r3[:], src[:], idx_t[:],
                            NI, NI, 128, transpose=False,
                            single_packet=SP, queue_num=q,
                        )
                        for m in range(K):
                            pt = ps.tile([128, CH], bf16, tag="pt", bufs=4,
                                         name="pt")
                            for t4 in range(4):
                                nc.tensor.transpose(
                                    pt[:, t4 * 128:(t4 + 1) * 128],
                                    r3[:, m * 4 + t4, :], ident[:])
                            nc.scalar.activation(
                                g_t[:, 0, m * CH:(m + 1) * CH], pt[:], AF.Copy)
                    for k in range(K):
                        nc.tensor.matmul(
                            py[:], wt_t[:, k, :],
                            g_t[:, 0, k * CH:(k + 1) * CH],
                            start=(k == 0), stop=(k == K - 1),
                        )
                    lo = chunk_lo(st)
                    nc.scalar.activation(y1raw[:, lo:lo + CH], py[:], AF.Copy)
                    if PARTS & 2:
                        a, b_ = owned_slice(st)
                        nc.vector.bn_stats(part[:, st, :], py[:, a:b_])
                    if (PARTS & 4) and do_gw and st < NST:
                        cs_t = wp.tile([15, CH], f32, tag="cs", bufs=4,
                                       name="cs_t")
                        cc_t = wp.tile([15, CH], f32, tag="cs", bufs=4,
                                       name="cc_t")
                        nc.sync.dma_start(cs_t[:], cs_d[:, st * CH:(st + 1) * CH])
                        nc.sync.dma_start(cc_t[:], cc_d[:, st * CH:(st + 1) * CH])
                        df_t = wp.tile([15, CH], bf16, tag="df", bufs=2,
                                       name="df_t")
                        nc.vector.tensor_tensor(out=df_t[:], in0=cs_t[:],
                                                in1=cc_t[:], op=OP.subtract)
                        sq_t = wp.tile([15, CH], bf16, tag="df", bufs=2,
                                       name="sq_t")
                        nc.vector.tensor_tensor(out=sq_t[:], in0=df_t[:],
                                                in1=df_t[:], op=OP.mult)
                        pd = ps.tile([5, CH], f32, tag="py", bufs=2, name="pd")
                        nc.tensor.matmul(pd[:], s15_t[:], sq_t[:],
                                         start=True, stop=True)
                        gwc = wp.tile([5, CH], bf16, tag="gwc", bufs=2,
                                      name="gwc")
                        nc.scalar.activation(gwc[:], pd[:], AF.Exp, scale=-0.5)
                        nc.sync.dma_start(gw_d[:, st * CH:(st + 1) * CH],
                                          gwc[:])

            # =====================================================
            def stats_phase(part, gi, si, cid):
                """bn partials -> AllReduce -> scale/bias into stv[:, si:si+2]"""
                agg = wp.tile([128, 2], f32, tag="st2", bufs=8, name="agg")
                nc.vector.bn_aggr(agg[:], part[:])
                msq = wp.tile([128, 1], f32, tag="st1", bufs=16, name="msq")
                nc.vector.tensor_tensor(out=msq[:], in0=agg[:, 0:1],
                                        in1=agg[:, 0:1], op=OP.mult)
                ari = wp.tile([128, 2], f32, tag="st2", bufs=8, name="ari")
                nc.vector.tensor_copy(ari[:, 0:1], agg[:, 0:1])
                nc.vector.tensor_tensor(out=ari[:, 1:2], in0=agg[:, 1:2],
                                        in1=msq[:], op=OP.add)
                nc.sync.dma_start(ar_in[cid][:], ari[:])
                nc.gpsimd.collective_compute(
                    "AllReduce", OP.add,
                    replica_groups=[list(range(NC_))],
                    ins=[ar_in[cid].opt()], outs=[ar_out[cid].opt()],
                )
                ars = wp.tile([128, 2], f32, tag="st2", bufs=8, name="ars")
                nc.sync.dma_start(ars[:], ar_out[cid][:])
                pm = wp.tile([128, 2], f32, tag="st2", bufs=8, name="pm")
                nc.vector.tensor_scalar(out=pm[:], in0=ars[:],
                                        scalar1=1.0 / NC_, scalar2=None,
                                        op0=OP.mult)
                m2 = wp.tile([128, 1], f32, tag="st1", bufs=16, name="m2")
                nc.vector.tensor_tensor(out=m2[:], in0=pm[:, 0:1],
                                        in1=pm[:, 0:1], op=OP.mult)
                var = wp.tile([128, 1], f32, tag="st1", bufs=16, name="var")
                nc.vector.tensor_tensor(out=var[:], in0=pm[:, 1:2],
                                        in1=m2[:], op=OP.subtract)
                vpe = wp.tile([128, 1], f32, tag="st1", bufs=16, name="vpe")
                nc.vector.tensor_scalar(out=vpe[:], in0=var[:],
                                        scalar1=float(EPS), scalar2=None,
                                        op0=OP.add)
                sd = wp.tile([128, 1], f32, tag="st1", bufs=16, name="sd")
                nc.scalar.activation(sd[:], vpe[:], AF.Sqrt)
                rs = wp.tile([128, 1], f32, tag="st1", bufs=16, name="rs")
                nc.vector.reciprocal(rs[:], sd[:])
                nc.vector.tensor_tensor(out=stv[:, si:si + 1],
                                        in0=gb_t[:, gi:gi + 1], in1=rs[:],
                                        op=OP.mult)
                ms = wp.tile([128, 1], f32, tag="st1", bufs=16, name="ms")
                nc.vector.tensor_tensor(out=ms[:], in0=pm[:, 0:1],
                                        in1=stv[:, si:si + 1], op=OP.mult)
                nc.vector.tensor_tensor(out=stv[:, si + 1:si + 2],
                                        in0=gb_t[:, gi + 1:gi + 2], in1=ms[:],
                                        op=OP.subtract)

            # =====================================================
            def bn_relu_pass(si):
                for st in range(NST + 1):
                    lo = st * CH
                    w = CH if st < NST else ME - M
                    nc.scalar.activation(
                        y1raw[:, lo:lo + w], y1raw[:, lo:lo + w], AF.Relu,
                        bias=stv[:, si + 1:si + 2], scale=stv[:, si:si + 1])

            # =====================================================
            def wconv_phase(wct_t, part):
                for st in range(NST):
                    base = st * CH
                    py2 = ps.tile([128, CH], f32, tag="py", bufs=2, name="py2")
                    gwt = wp.tile([5, CH], bf16, tag="gwl", bufs=3, name="gwt")
                    nc.sync.dma_start(gwt[:], gw_d[:, base:base + CH])
                    for ks in range(KS):
                        pgw = ps.tile([128, CH], f32, tag="pgw", bufs=2,
                                      name="pgw")
                        nc.tensor.matmul(pgw[:], rep5_t[:, ks, :],
                                         gwt[:],
                                         start=True, stop=True)
                        u_t = wp.tile([128, CH], bf16, tag="u", bufs=6,
                                      name="u_t")
                        nc.vector.tensor_tensor(
                            out=u_t[:], in0=pgw[:],
                            in1=y1raw[:, base + ks:base + ks + CH],
                            op=OP.mult)
                        nc.tensor.matmul(py2[:], wct_t[:, ks, :], u_t[:],
                                         start=(ks == 0), stop=(ks == KS - 1))
                    nc.vector.bn_stats(part[:, st, :], py2[:])
                    nc.scalar.activation(yraw2[:, base:base + CH], py2[:],
                                         AF.Copy)

            # =====================================================
            def dump(src_ap):
                # debug: cast-copy an AP [128, M] to the f32 output
                for st in range(NST):
                    sl = slice(st * CH, (st + 1) * CH)
                    nc.gpsimd.dma_start(out_d[:, sl], src_ap[:, sl])

            # layer 1
            if STAGE == 0:
                dump(yraw2)
            if STAGE >= 1:
                conv_gather_phase(xt_d, w1t_t, parts[0], do_gw=True)
            if STAGE == 1:
                dump(y1raw[:, 2:M + 2])
            if STAGE >= 2:
                stats_phase(parts[0], 0, 0, 0)
                bn_relu_pass(0)
            if STAGE == 2:
                dump(y1raw[:, 2:M + 2])
            if STAGE >= 3:
                wconv_phase(wc1t_t, parts[1])
            if STAGE == 3:
                dump(yraw2)
            if STAGE >= 4:
                stats_phase(parts[1], 2, 2, 1)

            # affine+relu, transpose, store halves, exchange
            for st in range(NST if STAGE >= 4 else 0):
                base = st * CH
                tmp = wp.tile([128, CH], bf16, tag="y2n", bufs=2, name="tmp")
                nc.scalar.activation(tmp[:], yraw2[:, base:base + CH], AF.Relu,
                                     bias=stv[:, 3:4], scale=stv[:, 2:3])
                stage = wp.tile([128, CH], bf16, tag="stage", bufs=2,
                                name="stage")
                for t4 in range(4):
                    ptr = ps.tile([128, 128], bf16, tag="py", bufs=2,
                                  name="ptr")
                    nc.tensor.transpose(ptr[:], tmp[:, t4 * 128:(t4 + 1) * 128],
                                        ident[:])
                    nc.vector.tensor_copy(stage[:, t4 * 128:(t4 + 1) * 128],
                                          ptr[:])
                nc.sync.dma_start(
                    d_my[base:base + CH, :].rearrange("(t p) o -> p t o", p=128),
                    stage[:].rearrange("p (t o) -> p t o", o=128))
            if STAGE >= 4:
                nc.gpsimd.collective_compute(
                    "AllGather", mybir.AluOpType.bypass,
                    replica_groups=[[2 * i, 2 * i + 1] for i in range(NC_ // 2)],
                    ins=[d_my.opt()], outs=[d_all.opt()],
                )
            if STAGE == 4:
                dump(yraw2)

            # layer 2
            if STAGE >= 5:
                conv_gather_phase(d_all, w2t_t, parts[2], do_gw=False)
                stats_phase(parts[2], 4, 4, 2)
                bn_relu_pass(4)
                wconv_phase(wc2t_t, parts[3])
                stats_phase(parts[3], 6, 6, 3)

            # final: out = relu(bn4(y4) + xres)
            for st in range(NST if STAGE >= 5 else 0):
                base = st * CH
                xr = wp.tile([128, CH], f32, tag="fw", bufs=4, name="xr")
                nc.sync.dma_start(xr[:], xres_d[:, base:base + CH])
                z = wp.tile([128, CH], f32, tag="fw", bufs=4, name="z")
                nc.vector.tensor_scalar(out=z[:], in0=yraw2[:, base:base + CH],
                                        scalar1=stv[:, 6:7],
                                        scalar2=stv[:, 7:8],
                                        op0=OP.mult, op1=OP.add)
                nc.vector.tensor_tensor(out=z[:], in0=z[:], in1=xr[:],
                                        op=OP.add)
                of = wp.tile([128, CH], f32, tag="fw", bufs=4, name="of")
                nc.scalar.activation(of[:], z[:], AF.Relu)
                nc.sync.dma_start(out_d[:, base:base + CH], of[:])

    nc.compile()
    return nc


def _wrap_idx(flat):
    """index i -> partition i%16, col i//16, replicated x8."""
    S = len(flat) // 16
    t16 = flat.astype(np.int16).reshape(S, 16).T
    return np.tile(t16, (8, 1))


def _prep_core_inputs(core, x, edge_index, coords, w1t, wc1t, w2t, wc2t,
                      rep5, s15, gbs):
    b, h = core // 2, core % 2
    xb = np.asarray(x[b], np.float32)                   # [C, N]
    xt = np.ascontiguousarray(xb.T).astype(BF16)        # [N, C]

    ei = np.asarray(edge_index[b])                      # [N, K]
    idx_chunks = np.zeros((NCHUNK, 128, K * CH // 16), np.int16)
    for st in range(NCHUNK):
        lo = st * CH if st < NCHUNK - 1 else ME - CH
        j = np.arange(lo, lo + CH)
        n = h * M - 2 + j
        valid = (n >= 0) & (n < N)
        nn = np.where(valid, n, 0)
        arr = ei[nn, :].T.astype(np.int16)              # [K, CH] k-major
        idx_chunks[st] = _wrap_idx(arr.reshape(-1))

    cb = np.asarray(coords[b], np.float32)              # [3, N]
    padded = np.full((3, N + 4), HUGE, np.float32)
    padded[:, 2:N + 2] = cb
    cs = np.empty((15, M), np.float32)
    for ks in range(KS):
        cs[ks * 3:(ks + 1) * 3] = padded[:, h * M + ks: h * M + ks + M]
    cc = np.tile(cb[:, h * M:(h + 1) * M], (KS, 1))

    return dict(
        xt=xt, idx=idx_chunks, cs=cs, cc=cc,
        xres=np.ascontiguousarray(xb[:, h * M:(h + 1) * M]),
        w1t=w1t, wc1t=wc1t, w2t=w2t, wc2t=wc2t,
        rep5=rep5, s15=s15, gb=gbs,
    )


def kernel(**inputs):
    from concourse import bass_utils

    if "nc" not in _CACHE:
        _CACHE["nc"] = _build_program()
    nc = _CACHE["nc"]

    x = np.asarray(inputs["x"], np.float32)
    edge_index = np.asarray(inputs["edge_index"])
    coords = np.asarray(inputs["coords"], np.float32)

    w1t = np.ascontiguousarray(
        np.transpose(np.asarray(inputs["w2d_1"], np.float32), (1, 2, 0))
    ).astype(BF16)
    wc1t = np.ascontiguousarray(
        np.transpose(np.asarray(inputs["wc_1"], np.float32), (1, 2, 0))
    ).astype(BF16)
    w2t = np.ascontiguousarray(
        np.transpose(np.asarray(inputs["w2d_2"], np.float32), (1, 2, 0))
    ).astype(BF16)
    wc2t = np.ascontiguousarray(
        np.transpose(np.asarray(inputs["wc_2"], np.float32), (1, 2, 0))
    ).astype(BF16)
    rep5 = np.zeros((5, KS, 128), np.float32)
    for ks in range(KS):
        rep5[ks, ks, :] = 1.0
    rep5 = rep5.astype(BF16)
    s15 = np.zeros((15, KS), np.float32)
    for r in range(15):
        s15[r, r // 3] = 1.0
    s15 = s15.astype(BF16)
    gbs = np.stack([
        np.asarray(inputs["g2d_1"], np.float32),
        np.asarray(inputs["b2d_1"], np.float32),
        np.asarray(inputs["g1d_1"], np.float32),
        np.asarray(inputs["b1d_1"], np.float32),
        np.asarray(inputs["g2d_2"], np.float32),
        np.asarray(inputs["b2d_2"], np.float32),
        np.asarray(inputs["g1d_2"], np.float32),
        np.asarray(inputs["b1d_2"], np.float32),
    ], axis=1)

    in_maps = [
        _prep_core_inputs(c, x, edge_index, coords, w1t, wc1t, w2t, wc2t,
                          rep5, s15, gbs)
        for c in range(NC_)
    ]
    res = bass_utils.run_bass_kernel_spmd(
        nc, in_maps, core_ids=list(range(NC_)),
        trace=_CACHE.get("trace", False),
    )
    _CACHE["last_results"] = res

    out = np.empty((B, C, N), np.float32)
    for c in range(NC_):
        b, h = c // 2, c % 2
        out[b, :, h * M:(h + 1) * M] = res.results[c]["out"]
    return out

